# revision 11
# baseline (speedup 1.0000x reference)
"""GAT (2-layer, 4-head) Bass kernel for Trainium2, data-parallel over 8 NeuronCores.

Math (per sample b, per attention instance with weights W, a = [a1; a2]):
    Wh = h @ W                      [N, F]
    s  = Wh @ a1   (per-dst-node i score part)
    t  = Wh @ a2   (per-src-node j score part)
    e[i,j]   = leaky_relu(s[i] + t[j], 0.2)
    att      = softmax_j(where(adj[i,j] > 0, e, -9e15))
    out[i]   = sum_j att[i,j] * Wh[j]

Key factorization: exp(lrelu(z)) = max(e^z, e^{0.2 z}) for z = s_i + t_j, so
    p[j,i] = m * max(e^{s_i} e^{t_j}, e^{0.2 s_i} e^{0.2 t_j})
           = m * e^{0.2 s_i} * max(e^{0.8 s_i + t_j}, e^{0.2 t_j})
The e^{0.2 s_i} factor is constant along the softmax axis (j) and cancels in
normalization, so the kernel computes only
    p'[j,i] = m[j,i] * max(G[i], r[j]) * F[j]
with G = e^{0.8 s} (one [128,N] ACT exp per instance, via the PE-replicated
s matmul), r = e^{-0.8 t}, F = e^{t} (tiny per-node columns). Per N^2-tile:
one DVE tensor_scalar (max with r, mult by F -- both per-partition scalar
APs, 4x DVE perf mode) and one native tensor_tensor mask multiply (2x perf
mode). No custom DVE ops, no per-tile ACT work.

Layout: all N x N tiles are [j(part), i(free)] so the final contraction over
j runs on the PE with the small [Wh | ones] block stationary; row 64 of the
[65, N] PSUM output is the softmax row-sum. Normalization: ACT copies the
rowsum row to SBUF f32, PE broadcasts it to [64, N] via an f32r ones-column
matmul, and the (otherwise idle) GPSIMD engine does the elementwise divide,
which also serves as the PSUM->SBUF move into h_cat^T / the elu input.
"""

import os
import sys

import numpy as np

if not os.path.isdir(os.path.join(os.path.dirname(os.path.abspath(__file__)), "concourse")):
    for _p in ("/opt/trn_rl_repo", os.path.expanduser("~/.axon_site/_ro/trn_rl_repo")):
        if os.path.isdir(_p) and _p not in sys.path:
            sys.path.append(_p)

import ml_dtypes  # noqa: E402

import concourse.bacc as bacc  # noqa: E402
import concourse.tile as tile  # noqa: E402
from concourse import mybir  # noqa: E402
from concourse.bass_utils import run_bass_kernel_spmd  # noqa: E402

BF16 = ml_dtypes.bfloat16

B, N, FIN, FH, H, FOUT = 16, 1024, 256, 64, 4, 64
NCORES = 8
SPC = B // NCORES  # samples per core
KT = FIN // 128    # k tiles (2)
JT = N // 128      # j tiles (8)
ALPHA = 0.2

F32 = mybir.dt.float32
F32R = mybir.dt.float32r
F16 = mybir.dt.float16
BF = mybir.dt.bfloat16
AF = mybir.ActivationFunctionType
OP = mybir.AluOpType
AX = mybir.AxisListType


def _gat_instance(nc, pools, maskT_sb, inst, emit_norm):
    """One attention instance (a head of L1, or L2).

    inst:
      rep(kt)  -> AP [128,128] bf16  column-replicated W@a1 (S matmul lhsT)
      rhs(kt)  -> AP [128,1024] bf16 x^T / h_cat^T k-tile
      wh(jt)   -> AP [128,65]  bf16  [Wh block | ones col] (att matmul lhsT)
      rcol(jt) -> AP [128,1] f32     exp(-0.8 t) column
      fcol(jt) -> AP [128,1] f32     exp(t) column
      ones64r  -> AP [1,64] f32r     for the rowsum broadcast matmul
    emit_norm(ot_ps, rsb_ps): consume the unnormalized [65,N] output + the
      [64,N] broadcast rowsum row (both PSUM).
    """
    work, psA, psO = pools["work"], pools["psA"], pools["psO"]
    workbig = pools["workbig"]

    # S_rep[p, i] = s[i] for all p (PE free-dim broadcast via replicated lhsT).
    sb_ps = psA.tile([128, N], F32, tag="big")
    for kt in range(KT):
        for ih in range(2):
            nc.tensor.matmul(
                sb_ps[:, ih * 512 : (ih + 1) * 512],
                inst["rep"](kt),
                inst["rhs"](kt)[:, ih * 512 : (ih + 1) * 512],
                start=(kt == 0),
                stop=(kt == KT - 1),
            )
    # G[p, i] = exp(0.8 * s[i])  (f16 SBUF; one big ACT pass per instance)
    g16 = work.tile([128, N], F16, tag="g16")
    nc.scalar.activation(g16, sb_ps, AF.Exp, scale=0.8)

    # p'[j,i] = max(G[i], r[j]) * F[j], then mask multiply (in-place, batched).
    pT = workbig.tile([128, JT, N], BF, tag="pt")
    for jt in range(JT):
        nc.vector.tensor_scalar(
            pT[:, jt, :], g16, inst["rcol"](jt), inst["fcol"](jt), OP.max, OP.mult
        )
    for ih in range(2):
        half = slice(ih * (JT // 2), (ih + 1) * (JT // 2))
        nc.vector.tensor_tensor(
            pT[:, half, :], pT[:, half, :], maskT_sb[:, half, :], OP.mult
        )

    # O^T[f, i] (+ rowsum row 64) accumulated over j-tiles.
    ot_ps = psO.tile([FH + 1, N], F32, tag="ot")
    for jt in range(JT):
        for ih in range(2):
            nc.tensor.matmul(
                ot_ps[:, ih * 512 : (ih + 1) * 512],
                inst["wh"](jt),
                pT[:, jt, ih * 512 : (ih + 1) * 512],
                start=(jt == 0),
                stop=(jt == JT - 1),
            )

    # Deferred tail: reciprocal of the rowsum row, GPSIMD partition-broadcast,
    # GPSIMD normalize multiply (which is also the PSUM->SBUF move of O^T).
    # Returned as a closure so the caller can emit it AFTER the next
    # instance's main body -- otherwise every engine idles on the chain.
    def finish():
        ot_sb = work.tile([FH, N], BF, tag="otsb")
        nc.scalar.copy(ot_sb, ot_ps[0:FH, :])
        rs = work.tile([1, N], F32, tag="rs")
        nc.scalar.copy(rs, ot_ps[FH : FH + 1, :])
        rb = work.tile([1, N], F32, tag="rb")
        nc.vector.reciprocal_approx_fast(out=rb, in_=rs)
        rbc_sb = work.tile([FH, N], F32, tag="rbc")
        nc.gpsimd.partition_broadcast(rbc_sb, rb)
        emit_norm(ot_sb, rbc_sb)

    return finish


def _build_nc():
    nc = bacc.Bacc()

    xT_d = nc.declare_dram_parameter("xT", [SPC, KT, 128, N], BF, isOutput=False)
    maskT_d = nc.declare_dram_parameter("maskT", [SPC, JT, 128, N], BF, isOutput=False)
    wbig1_d = nc.declare_dram_parameter("wbig1", [KT, 128, H * 65 + H], BF, isOutput=False)
    warep1_d = nc.declare_dram_parameter("warep1", [KT, 128, H * 128], BF, isOutput=False)
    wbig2_d = nc.declare_dram_parameter("wbig2", [KT, 128, 66], BF, isOutput=False)
    warep2_d = nc.declare_dram_parameter("warep2", [KT, 128, 128], BF, isOutput=False)
    out_d = nc.declare_dram_parameter("out", [SPC, FOUT], F32, isOutput=True)

    with tile.TileContext(nc) as tc:
        with (
            tc.tile_pool(name="const", bufs=1) as constp,
            tc.tile_pool(name="samp", bufs=2) as samp,
            tc.tile_pool(name="workbig", bufs=2) as workbig,
            tc.tile_pool(name="work", bufs=3) as work,
            tc.tile_pool(name="tail", bufs=1) as tail,
            tc.tile_pool(name="psA", bufs=2, space="PSUM") as psA,
            tc.tile_pool(name="psO", bufs=2, space="PSUM") as psO,
        ):
            pools = {"work": work, "workbig": workbig, "psA": psA, "psO": psO}

            wbig1_sb = constp.tile([128, KT, H * 65 + H], BF)
            warep1_sb = constp.tile([128, KT, H * 128], BF)
            wbig2_sb = constp.tile([128, KT, 66], BF)
            warep2_sb = constp.tile([128, KT, 128], BF)
            for kt in range(KT):
                nc.sync.dma_start(out=wbig1_sb[:, kt, :], in_=wbig1_d[kt])
                nc.sync.dma_start(out=warep1_sb[:, kt, :], in_=warep1_d[kt])
                nc.sync.dma_start(out=wbig2_sb[:, kt, :], in_=wbig2_d[kt])
                nc.sync.dma_start(out=warep2_sb[:, kt, :], in_=warep2_d[kt])

            pending_sample_tail = None
            for s in range(SPC):
                xT_sb = samp.tile([128, KT, N], BF, tag="xt")
                for kt in range(KT):
                    nc.sync.dma_start(out=xT_sb[:, kt, :], in_=xT_d[s, kt])
                maskT_sb = samp.tile([128, JT, N], BF, tag="mask")
                for jt in range(JT):
                    nc.sync.dma_start(out=maskT_sb[:, jt, :], in_=maskT_d[s, jt])

                # ---- L1 Wh for all 4 heads (+ t columns) ----
                whsb1 = samp.tile([128, JT, H * 65], BF, tag="whsb1")
                tc1 = samp.tile([128, JT, H], F32, tag="tc1")
                for jt in range(JT):
                    wm_ps = psA.tile([128, H * 65 + H], F32, tag="big")
                    for kt in range(KT):
                        nc.tensor.matmul(
                            wm_ps,
                            xT_sb[:, kt, jt * 128 : (jt + 1) * 128],
                            wbig1_sb[:, kt, :],
                            start=(kt == 0),
                            stop=(kt == KT - 1),
                        )
                    nc.scalar.copy(whsb1[:, jt, :], wm_ps[:, 0 : H * 65])
                    nc.vector.memset(whsb1[:, jt, FH : H * 65 : 65], 1.0)
                    nc.vector.tensor_copy(tc1[:, jt, :], wm_ps[:, H * 65 : H * 65 + H])
                # r = exp(-0.8 t), F = exp(t) for all heads/j-tiles at once.
                rc1 = samp.tile([128, JT, H], F32, tag="rc1")
                fc1 = samp.tile([128, JT, H], F32, tag="fc1")
                nc.scalar.activation(rc1, tc1, AF.Exp, scale=-0.8)
                nc.scalar.activation(fc1, tc1, AF.Exp, scale=1.0)

                if pending_sample_tail is not None:
                    pending_sample_tail()
                    pending_sample_tail = None

                # ---- L1 attention, 4 heads -> h_cat^T ----
                hcatT = samp.tile([128, KT, N], BF, tag="hcat")

                pending = None
                for h in range(H):
                    def emit_l1(ot_sb, rbc_sb, h=h):
                        dst = hcatT[(h % 2) * 64 : (h % 2) * 64 + 64, h // 2, :]
                        nc.gpsimd.tensor_tensor(dst, ot_sb, rbc_sb, OP.mult)

                    fin = _gat_instance(
                        nc,
                        pools,
                        maskT_sb,
                        {
                            "rep": lambda kt, h=h: warep1_sb[:, kt, h * 128 : (h + 1) * 128],
                            "rhs": lambda kt: xT_sb[:, kt, :],
                            "wh": lambda jt, h=h: whsb1[:, jt, h * 65 : (h + 1) * 65],
                            "rcol": lambda jt, h=h: rc1[:, jt, h : h + 1],
                            "fcol": lambda jt, h=h: fc1[:, jt, h : h + 1],
                        },
                        emit_l1,
                    )
                    if pending is not None:
                        pending()
                    pending = fin
                pending()

                # ---- L2 Wh ----
                whsb2 = samp.tile([128, JT, 65], BF, tag="whsb2")
                tc2 = samp.tile([128, JT, 1], F32, tag="tc2")
                for jt in range(JT):
                    wm_ps = psA.tile([128, 66], F32, tag="big")
                    for kt in range(KT):
                        nc.tensor.matmul(
                            wm_ps,
                            hcatT[:, kt, jt * 128 : (jt + 1) * 128],
                            wbig2_sb[:, kt, :],
                            start=(kt == 0),
                            stop=(kt == KT - 1),
                        )
                    nc.vector.tensor_copy(whsb2[:, jt, 0:FOUT], wm_ps[:, 0:FOUT])
                    nc.vector.memset(whsb2[:, jt, FOUT : FOUT + 1], 1.0)
                    nc.vector.tensor_copy(tc2[:, jt, :], wm_ps[:, 65:66])
                rc2 = samp.tile([128, JT, 1], F32, tag="rc2")
                fc2 = samp.tile([128, JT, 1], F32, tag="fc2")
                nc.scalar.activation(rc2, tc2, AF.Exp, scale=-0.8)
                nc.scalar.activation(fc2, tc2, AF.Exp, scale=1.0)

                # ---- L2 attention + elu + mean ----
                o2n = tail.tile([FH, N], F32, tag="o2n")

                def emit_l2(ot_sb, rbc_sb, o2n=o2n):
                    nc.gpsimd.tensor_tensor(o2n, ot_sb, rbc_sb, OP.mult)

                fin2 = _gat_instance(
                    nc,
                    pools,
                    maskT_sb,
                    {
                        "rep": lambda kt: warep2_sb[:, kt, :],
                        "rhs": lambda kt: hcatT[:, kt, :],
                        "wh": lambda jt: whsb2[:, jt, :],
                        "rcol": lambda jt: rc2[:, jt, :],
                        "fcol": lambda jt: fc2[:, jt, :],
                    },
                    emit_l2,
                )
                def sample_tail(s=s, fin2=fin2, o2n=o2n):
                    fin2()
                    # elu(x) = relu(x) + min(exp(x), 1) - 1; the constant -1
                    # is folded into the post-reduce scale: mean = sum/N - 1.
                    ex = tail.tile([FH, N], F32, tag="ex")
                    nc.scalar.activation(ex, o2n, AF.Exp)
                    bmax = tail.tile([FH, N], F32, tag="bmax")
                    nc.scalar.activation(bmax, o2n, AF.Relu)
                    eluv = tail.tile([FH, N], F32, tag="eluv")
                    red = tail.tile([FH, 1], F32, tag="red")
                    nc.vector.scalar_tensor_tensor(
                        eluv, ex, 1.0, bmax, OP.min, OP.add, accum_out=red
                    )
                    outc = tail.tile([FH, 1], F32, tag="outc")
                    nc.gpsimd.tensor_scalar(outc, red, 1.0 / N, -1.0, OP.mult, OP.add)
                    nc.sync.dma_start(
                        out=out_d[s].rearrange("(f a) -> f a", a=1), in_=outc
                    )

                pending_sample_tail = sample_tail

            pending_sample_tail()

    nc.finalize()
    return nc


_NC_CACHE = None


def _prep_host(x, adj, W_heads, a_heads, W_out, a_out):
    xT = np.ascontiguousarray(np.asarray(x, np.float32).transpose(0, 2, 1)).astype(BF16)
    xT = xT.reshape(B, KT, 128, N)
    maskT = (np.asarray(adj) > 0).transpose(0, 2, 1).astype(BF16)  # [B, j, i]
    maskT = np.ascontiguousarray(maskT).reshape(B, JT, 128, N)

    W_heads = np.asarray(W_heads, np.float32)
    a_heads = np.asarray(a_heads, np.float32)
    W_out = np.asarray(W_out, np.float32)
    a_out = np.asarray(a_out, np.float32)

    wbig1 = np.zeros((FIN, H * 65 + H), dtype=np.float32)
    warep1 = np.zeros((FIN, H * 128), dtype=np.float32)
    for h in range(H):
        Wh_ = W_heads[h]
        wbig1[:, h * 65 : h * 65 + FH] = Wh_
        wbig1[:, H * 65 + h] = Wh_ @ a_heads[h, FH:, 0]
        warep1[:, h * 128 : (h + 1) * 128] = (Wh_ @ a_heads[h, :FH, 0])[:, None]
    wbig2 = np.zeros((FIN, 66), dtype=np.float32)
    wbig2[:, 0:FOUT] = W_out
    wbig2[:, 65] = W_out @ a_out[FOUT:, 0]
    warep2 = np.repeat((W_out @ a_out[:FOUT, 0])[:, None], 128, axis=1)

    shared = {
        "wbig1": wbig1.astype(BF16).reshape(KT, 128, H * 65 + H),
        "warep1": warep1.astype(BF16).reshape(KT, 128, H * 128),
        "wbig2": wbig2.astype(BF16).reshape(KT, 128, 66),
        "warep2": warep2.astype(BF16).reshape(KT, 128, 128),
    }
    in_maps = []
    for c in range(NCORES):
        sl = slice(c * SPC, (c + 1) * SPC)
        m = {"xT": np.ascontiguousarray(xT[sl]), "maskT": np.ascontiguousarray(maskT[sl])}
        m.update(shared)
        in_maps.append(m)
    return in_maps


def kernel(x, adj, W_heads, a_heads, W_out, a_out, _trace=False):
    global _NC_CACHE
    if _NC_CACHE is None:
        _NC_CACHE = _build_nc()
    nc = _NC_CACHE
    in_maps = _prep_host(x, adj, W_heads, a_heads, W_out, a_out)
    res = run_bass_kernel_spmd(nc, in_maps, core_ids=list(range(NCORES)), trace=_trace)
    out = np.concatenate([res.results[c]["out"] for c in range(NCORES)], axis=0)
    if _trace:
        kernel._last_results = res
    return out.astype(np.float32)


# revision 12
# speedup vs baseline: 1.4763x; 1.4763x over previous
"""GAT (2-layer, 4-head) Bass kernel for Trainium2, data-parallel over 8 NeuronCores.

Math (per sample b, per attention instance with weights W, a = [a1; a2]):
    Wh = h @ W                      [N, F]
    s  = Wh @ a1   (per-dst-node i score part)
    t  = Wh @ a2   (per-src-node j score part)
    e[i,j]   = leaky_relu(s[i] + t[j], 0.2)
    att      = softmax_j(where(adj[i,j] > 0, e, -9e15))
    out[i]   = sum_j att[i,j] * Wh[j]

Key factorization: exp(lrelu(z)) = max(e^z, e^{0.2 z}) for z = s_i + t_j, so
    p[j,i] = m * max(e^{s_i} e^{t_j}, e^{0.2 s_i} e^{0.2 t_j})
           = m * e^{0.2 s_i} * max(e^{0.8 s_i + t_j}, e^{0.2 t_j})
The e^{0.2 s_i} factor is constant along the softmax axis (j) and cancels in
normalization, so the kernel computes only
    p'[j,i] = m[j,i] * max(G[i], r[j]) * F[j]
with G = e^{0.8 s} (one [128,N] ACT exp per instance, via the PE-replicated
s matmul), r = e^{-0.8 t}, F = e^{t} (tiny per-node columns). Per N^2-tile:
one DVE tensor_scalar (max with r, mult by F -- both per-partition scalar
APs, 4x DVE perf mode) and one native tensor_tensor mask multiply (2x perf
mode). No custom DVE ops, no per-tile ACT work.

Layout: all N x N tiles are [j(part), i(free)] so the final contraction over
j runs on the PE with the small [Wh | ones] block stationary; row 64 of the
[65, N] PSUM output is the softmax row-sum. Normalization: ACT copies the
rowsum row to SBUF f32, PE broadcasts it to [64, N] via an f32r ones-column
matmul, and the (otherwise idle) GPSIMD engine does the elementwise divide,
which also serves as the PSUM->SBUF move into h_cat^T / the elu input.
"""

import os
import sys

import numpy as np

if not os.path.isdir(os.path.join(os.path.dirname(os.path.abspath(__file__)), "concourse")):
    for _p in ("/opt/trn_rl_repo", os.path.expanduser("~/.axon_site/_ro/trn_rl_repo")):
        if os.path.isdir(_p) and _p not in sys.path:
            sys.path.append(_p)

import ml_dtypes  # noqa: E402

import concourse.bacc as bacc  # noqa: E402
import concourse.tile as tile  # noqa: E402
from concourse import mybir  # noqa: E402
from concourse.bass_utils import run_bass_kernel_spmd  # noqa: E402

BF16 = ml_dtypes.bfloat16

B, N, FIN, FH, H, FOUT = 16, 1024, 256, 64, 4, 64
NCORES = 8
SPC = B // NCORES  # samples per core
KT = FIN // 128    # k tiles (2)
JT = N // 128      # j tiles (8)
ALPHA = 0.2

F32 = mybir.dt.float32
F32R = mybir.dt.float32r
F16 = mybir.dt.float16
BF = mybir.dt.bfloat16
AF = mybir.ActivationFunctionType
OP = mybir.AluOpType
AX = mybir.AxisListType


def _gat_instance(nc, pools, maskT_sb, inst, emit_norm):
    """One attention instance (a head of L1, or L2).

    inst:
      rep(kt)  -> AP [128,128] bf16  column-replicated W@a1 (S matmul lhsT)
      rhs(kt)  -> AP [128,1024] bf16 x^T / h_cat^T k-tile
      wh(jt)   -> AP [128,65]  bf16  [Wh block | ones col] (att matmul lhsT)
      rcol(jt) -> AP [128,1] f32     exp(-0.8 t) column
      fcol(jt) -> AP [128,1] f32     exp(t) column
      ones64r  -> AP [1,64] f32r     for the rowsum broadcast matmul
    emit_norm(ot_ps, rsb_ps): consume the unnormalized [65,N] output + the
      [64,N] broadcast rowsum row (both PSUM).
    """
    work, psA, psO = pools["work"], pools["psA"], pools["psO"]
    workbig = pools["workbig"]

    # S_rep[p, i] = s[i] for all p (PE free-dim broadcast via replicated lhsT).
    sb_ps = psA.tile([128, N], F32, tag="big")
    for kt in range(KT):
        for ih in range(2):
            nc.tensor.matmul(
                sb_ps[:, ih * 512 : (ih + 1) * 512],
                inst["rep"](kt),
                inst["rhs"](kt)[:, ih * 512 : (ih + 1) * 512],
                start=(kt == 0),
                stop=(kt == KT - 1),
            )
    # G[p, i] = exp(0.8 * s[i])  (f16 SBUF; one big ACT pass per instance)
    g16 = work.tile([128, N], BF, tag="g16")
    nc.scalar.activation(g16, sb_ps, AF.Exp, scale=0.8)

    # p'[j,i] = max(G[i], r[j]) * F[j], then mask multiply (in-place, batched).
    pT = workbig.tile([128, JT, N], BF, tag="pt")
    for jt in range(JT):
        nc.vector.tensor_scalar(
            pT[:, jt, :], g16, inst["rcol"](jt), inst["fcol"](jt), OP.max, OP.mult
        )
    for ih in range(2):
        half = slice(ih * (JT // 2), (ih + 1) * (JT // 2))
        nc.vector.tensor_tensor(
            pT[:, half, :], pT[:, half, :], maskT_sb[:, half, :], OP.mult
        )

    # O^T[f, i] (+ rowsum row 64) accumulated over j-tiles.
    ot_ps = psO.tile([FH + 1, N], F32, tag="ot")
    for jt in range(JT):
        for ih in range(2):
            nc.tensor.matmul(
                ot_ps[:, ih * 512 : (ih + 1) * 512],
                inst["wh"](jt),
                pT[:, jt, ih * 512 : (ih + 1) * 512],
                start=(jt == 0),
                stop=(jt == JT - 1),
            )

    # Deferred tail: reciprocal of the rowsum row, PE broadcast to [64, N],
    # DVE normalize multiply. Returned as a closure so the caller can emit it
    # AFTER the next instance's main body -- otherwise every engine idles.
    def finish():
        ot_sb = work.tile([FH, N], BF, tag="otsb")
        nc.scalar.copy(ot_sb, ot_ps[0:FH, :])
        rs = work.tile([1, N], F32, tag="rs")
        nc.scalar.copy(rs, ot_ps[FH : FH + 1, :])
        rb = work.tile([1, N], F32, tag="rb")
        nc.vector.reciprocal_approx_fast(out=rb, in_=rs)
        rb_bf = work.tile([1, N], BF, tag="rbbf")
        nc.scalar.copy(rb_bf, rb)
        rbc_ps = psA.tile([FH, N], F32, tag="big")
        for ih in range(2):
            nc.tensor.matmul(
                rbc_ps[:, ih * 512 : (ih + 1) * 512],
                inst["ones64"],
                rb_bf[:, ih * 512 : (ih + 1) * 512],
                start=True,
                stop=True,
            )
        emit_norm(ot_sb, rbc_ps)

    return finish


def _build_nc():
    nc = bacc.Bacc()

    xT_d = nc.declare_dram_parameter("xT", [SPC, KT, 128, N], BF, isOutput=False)
    maskT_d = nc.declare_dram_parameter("maskT", [SPC, JT, 128, N], BF, isOutput=False)
    wbig1_d = nc.declare_dram_parameter("wbig1", [KT, 128, H * 65 + H], BF, isOutput=False)
    warep1_d = nc.declare_dram_parameter("warep1", [KT, 128, H * 128], BF, isOutput=False)
    wbig2_d = nc.declare_dram_parameter("wbig2", [KT, 128, 66], BF, isOutput=False)
    warep2_d = nc.declare_dram_parameter("warep2", [KT, 128, 128], BF, isOutput=False)
    out_d = nc.declare_dram_parameter("out", [SPC, FOUT], F32, isOutput=True)

    with tile.TileContext(nc) as tc:
        with (
            tc.tile_pool(name="const", bufs=1) as constp,
            tc.tile_pool(name="samp", bufs=2) as samp,
            tc.tile_pool(name="workbig", bufs=2) as workbig,
            tc.tile_pool(name="work", bufs=3) as work,
            tc.tile_pool(name="tail", bufs=1) as tail,
            tc.tile_pool(name="psA", bufs=2, space="PSUM") as psA,
            tc.tile_pool(name="psO", bufs=2, space="PSUM") as psO,
        ):
            pools = {"work": work, "workbig": workbig, "psA": psA, "psO": psO}

            wbig1_sb = constp.tile([128, KT, H * 65 + H], BF)
            warep1_sb = constp.tile([128, KT, H * 128], BF)
            wbig2_sb = constp.tile([128, KT, 66], BF)
            warep2_sb = constp.tile([128, KT, 128], BF)
            ones64_sb = constp.tile([1, FH], BF)
            for kt in range(KT):
                nc.sync.dma_start(out=wbig1_sb[:, kt, :], in_=wbig1_d[kt])
                nc.sync.dma_start(out=warep1_sb[:, kt, :], in_=warep1_d[kt])
                nc.sync.dma_start(out=wbig2_sb[:, kt, :], in_=wbig2_d[kt])
                nc.sync.dma_start(out=warep2_sb[:, kt, :], in_=warep2_d[kt])
            nc.vector.memset(ones64_sb, 1.0)

            pending_sample_tail = None
            for s in range(SPC):
                xT_sb = samp.tile([128, KT, N], BF, tag="xt")
                for kt in range(KT):
                    nc.sync.dma_start(out=xT_sb[:, kt, :], in_=xT_d[s, kt])
                maskT_sb = samp.tile([128, JT, N], BF, tag="mask")
                for jt in range(JT):
                    nc.sync.dma_start(out=maskT_sb[:, jt, :], in_=maskT_d[s, jt])

                # ---- L1 Wh for all 4 heads (+ t columns) ----
                whsb1 = samp.tile([128, JT, H * 65], BF, tag="whsb1")
                tc1 = samp.tile([128, JT, H], F32, tag="tc1")
                for jt in range(JT):
                    wm_ps = psA.tile([128, H * 65 + H], F32, tag="big")
                    for kt in range(KT):
                        nc.tensor.matmul(
                            wm_ps,
                            xT_sb[:, kt, jt * 128 : (jt + 1) * 128],
                            wbig1_sb[:, kt, :],
                            start=(kt == 0),
                            stop=(kt == KT - 1),
                        )
                    nc.scalar.copy(whsb1[:, jt, :], wm_ps[:, 0 : H * 65])
                    nc.vector.memset(whsb1[:, jt, FH : H * 65 : 65], 1.0)
                    nc.vector.tensor_copy(tc1[:, jt, :], wm_ps[:, H * 65 : H * 65 + H])
                # r = exp(-0.8 t), F = exp(t) for all heads/j-tiles at once.
                rc1 = samp.tile([128, JT, H], F32, tag="rc1")
                fc1 = samp.tile([128, JT, H], F32, tag="fc1")
                nc.scalar.activation(rc1, tc1, AF.Exp, scale=-0.8)
                nc.scalar.activation(fc1, tc1, AF.Exp, scale=1.0)

                if pending_sample_tail is not None:
                    pending_sample_tail()
                    pending_sample_tail = None

                # ---- L1 attention, 4 heads -> h_cat^T ----
                hcatT = samp.tile([128, KT, N], BF, tag="hcat")

                pending = None
                for h in range(H):
                    def emit_l1(ot_sb, rbc_ps, h=h):
                        dst = hcatT[(h % 2) * 64 : (h % 2) * 64 + 64, h // 2, :]
                        nc.vector.tensor_tensor(dst, ot_sb, rbc_ps, OP.mult)

                    fin = _gat_instance(
                        nc,
                        pools,
                        maskT_sb,
                        {
                            "rep": lambda kt, h=h: warep1_sb[:, kt, h * 128 : (h + 1) * 128],
                            "rhs": lambda kt: xT_sb[:, kt, :],
                            "wh": lambda jt, h=h: whsb1[:, jt, h * 65 : (h + 1) * 65],
                            "rcol": lambda jt, h=h: rc1[:, jt, h : h + 1],
                            "fcol": lambda jt, h=h: fc1[:, jt, h : h + 1],
                            "ones64": ones64_sb,
                        },
                        emit_l1,
                    )
                    if pending is not None:
                        pending()
                    pending = fin
                pending()

                # ---- L2 Wh ----
                whsb2 = samp.tile([128, JT, 65], BF, tag="whsb2")
                tc2 = samp.tile([128, JT, 1], F32, tag="tc2")
                for jt in range(JT):
                    wm_ps = psA.tile([128, 66], F32, tag="big")
                    for kt in range(KT):
                        nc.tensor.matmul(
                            wm_ps,
                            hcatT[:, kt, jt * 128 : (jt + 1) * 128],
                            wbig2_sb[:, kt, :],
                            start=(kt == 0),
                            stop=(kt == KT - 1),
                        )
                    nc.vector.tensor_copy(whsb2[:, jt, 0:FOUT], wm_ps[:, 0:FOUT])
                    nc.vector.memset(whsb2[:, jt, FOUT : FOUT + 1], 1.0)
                    nc.vector.tensor_copy(tc2[:, jt, :], wm_ps[:, 65:66])
                rc2 = samp.tile([128, JT, 1], F32, tag="rc2")
                fc2 = samp.tile([128, JT, 1], F32, tag="fc2")
                nc.scalar.activation(rc2, tc2, AF.Exp, scale=-0.8)
                nc.scalar.activation(fc2, tc2, AF.Exp, scale=1.0)

                # ---- L2 attention + elu + mean ----
                o2n = tail.tile([FH, N], F32, tag="o2n")

                def emit_l2(ot_sb, rbc_ps, o2n=o2n):
                    nc.vector.tensor_tensor(o2n, ot_sb, rbc_ps, OP.mult)

                fin2 = _gat_instance(
                    nc,
                    pools,
                    maskT_sb,
                    {
                        "rep": lambda kt: warep2_sb[:, kt, :],
                        "rhs": lambda kt: hcatT[:, kt, :],
                        "wh": lambda jt: whsb2[:, jt, :],
                        "rcol": lambda jt: rc2[:, jt, :],
                        "fcol": lambda jt: fc2[:, jt, :],
                        "ones64": ones64_sb,
                    },
                    emit_l2,
                )
                def sample_tail(s=s, fin2=fin2, o2n=o2n):
                    fin2()
                    # elu(x) = relu(x) + min(exp(x), 1) - 1; the constant -1
                    # is folded into the post-reduce scale: mean = sum/N - 1.
                    ex = tail.tile([FH, N], F32, tag="ex")
                    nc.scalar.activation(ex, o2n, AF.Exp)
                    bmax = tail.tile([FH, N], F32, tag="bmax")
                    nc.scalar.activation(bmax, o2n, AF.Relu)
                    eluv = tail.tile([FH, N], F32, tag="eluv")
                    red = tail.tile([FH, 1], F32, tag="red")
                    nc.vector.scalar_tensor_tensor(
                        eluv, ex, 1.0, bmax, OP.min, OP.add, accum_out=red
                    )
                    outc = tail.tile([FH, 1], F32, tag="outc")
                    nc.vector.tensor_scalar(outc, red, 1.0 / N, -1.0, OP.mult, OP.add)
                    nc.sync.dma_start(
                        out=out_d[s].rearrange("(f a) -> f a", a=1), in_=outc
                    )

                pending_sample_tail = sample_tail

            pending_sample_tail()

    nc.finalize()
    return nc


_NC_CACHE = None


def _prep_host(x, adj, W_heads, a_heads, W_out, a_out):
    xT = np.ascontiguousarray(np.asarray(x, np.float32).transpose(0, 2, 1)).astype(BF16)
    xT = xT.reshape(B, KT, 128, N)
    maskT = (np.asarray(adj) > 0).transpose(0, 2, 1).astype(BF16)  # [B, j, i]
    maskT = np.ascontiguousarray(maskT).reshape(B, JT, 128, N)

    W_heads = np.asarray(W_heads, np.float32)
    a_heads = np.asarray(a_heads, np.float32)
    W_out = np.asarray(W_out, np.float32)
    a_out = np.asarray(a_out, np.float32)

    wbig1 = np.zeros((FIN, H * 65 + H), dtype=np.float32)
    warep1 = np.zeros((FIN, H * 128), dtype=np.float32)
    for h in range(H):
        Wh_ = W_heads[h]
        wbig1[:, h * 65 : h * 65 + FH] = Wh_
        wbig1[:, H * 65 + h] = Wh_ @ a_heads[h, FH:, 0]
        warep1[:, h * 128 : (h + 1) * 128] = (Wh_ @ a_heads[h, :FH, 0])[:, None]
    wbig2 = np.zeros((FIN, 66), dtype=np.float32)
    wbig2[:, 0:FOUT] = W_out
    wbig2[:, 65] = W_out @ a_out[FOUT:, 0]
    warep2 = np.repeat((W_out @ a_out[:FOUT, 0])[:, None], 128, axis=1)

    shared = {
        "wbig1": wbig1.astype(BF16).reshape(KT, 128, H * 65 + H),
        "warep1": warep1.astype(BF16).reshape(KT, 128, H * 128),
        "wbig2": wbig2.astype(BF16).reshape(KT, 128, 66),
        "warep2": warep2.astype(BF16).reshape(KT, 128, 128),
    }
    in_maps = []
    for c in range(NCORES):
        sl = slice(c * SPC, (c + 1) * SPC)
        m = {"xT": np.ascontiguousarray(xT[sl]), "maskT": np.ascontiguousarray(maskT[sl])}
        m.update(shared)
        in_maps.append(m)
    return in_maps


def kernel(x, adj, W_heads, a_heads, W_out, a_out, _trace=False):
    global _NC_CACHE
    if _NC_CACHE is None:
        _NC_CACHE = _build_nc()
    nc = _NC_CACHE
    in_maps = _prep_host(x, adj, W_heads, a_heads, W_out, a_out)
    res = run_bass_kernel_spmd(nc, in_maps, core_ids=list(range(NCORES)), trace=_trace)
    out = np.concatenate([res.results[c]["out"] for c in range(NCORES)], axis=0)
    if _trace:
        kernel._last_results = res
    return out.astype(np.float32)


# revision 15
# speedup vs baseline: 1.5268x; 1.0342x over previous
"""GAT (2-layer, 4-head) Bass kernel for Trainium2, data-parallel over 8 NeuronCores.

Math (per sample b, per attention instance with weights W, a = [a1; a2]):
    Wh = h @ W                      [N, F]
    s  = Wh @ a1   (per-dst-node i score part)
    t  = Wh @ a2   (per-src-node j score part)
    e[i,j]   = leaky_relu(s[i] + t[j], 0.2)
    att      = softmax_j(where(adj[i,j] > 0, e, -9e15))
    out[i]   = sum_j att[i,j] * Wh[j]

Key factorization: exp(lrelu(z)) = max(e^z, e^{0.2 z}) for z = s_i + t_j, so
    p[j,i] = m * max(e^{s_i} e^{t_j}, e^{0.2 s_i} e^{0.2 t_j})
           = m * e^{0.2 s_i} * max(e^{0.8 s_i + t_j}, e^{0.2 t_j})
The e^{0.2 s_i} factor is constant along the softmax axis (j) and cancels in
normalization, so the kernel computes only
    p'[j,i] = m[j,i] * max(G[i], r[j]) * F[j]
with G = e^{0.8 s} (one [128,N] ACT exp per instance, via the PE-replicated
s matmul), r = e^{-0.8 t}, F = e^{t} (tiny per-node columns). Per N^2-tile:
one DVE tensor_scalar (max with r-column, mult by F-column) and one native
tensor_tensor mask multiply (2x DVE perf mode). No custom DVE ops, no
per-tile ACT work.

Attention-apply orientation: the contraction over j runs with the p' tile
[j, i-chunk] as the PE stationary and the small [Wh | ones] block moving, so
the output lands as O[i, blk, f] with the softmax row-sum in column 64 --
i.e. BOTH the output and the row-sum are per-i-PARTITION. The reciprocal
then runs on a [128, 8] column (free-size 8, ~100x cheaper than a [1, N]
row) and normalization fuses into the PSUM->SBUF copy as ACT Copy with a
per-partition scale AP. Layer-1 heads are transposed back to h_cat^T
[feat, i] with PE transpose blocks; layer 2 consumes O[i, f] directly
(elu elementwise, mean over nodes via a PE ones-column contraction).
"""

import os
import sys

import numpy as np

if not os.path.isdir(os.path.join(os.path.dirname(os.path.abspath(__file__)), "concourse")):
    for _p in ("/opt/trn_rl_repo", os.path.expanduser("~/.axon_site/_ro/trn_rl_repo")):
        if os.path.isdir(_p) and _p not in sys.path:
            sys.path.append(_p)

import ml_dtypes  # noqa: E402

import concourse.bacc as bacc  # noqa: E402
import concourse.tile as tile  # noqa: E402
from concourse import mybir  # noqa: E402
from concourse.bass_utils import run_bass_kernel_spmd  # noqa: E402

BF16 = ml_dtypes.bfloat16

B, N, FIN, FH, H, FOUT = 16, 1024, 256, 64, 4, 64
NCORES = 8
SPC = B // NCORES  # samples per core
KT = FIN // 128    # k tiles (2)
JT = N // 128      # j tiles (8)
IB = N // 128      # i chunks (8)
ALPHA = 0.2

F32 = mybir.dt.float32
F16 = mybir.dt.float16
BF = mybir.dt.bfloat16
AF = mybir.ActivationFunctionType
OP = mybir.AluOpType
AX = mybir.AxisListType


def _gat_instance(nc, pools, maskT_sb, inst, out_dt, emit_out):
    """One attention instance (a head of L1, or L2).

    inst:
      rep(kt)  -> AP [128,128] bf16  column-replicated W@a1 (S matmul lhsT)
      rhs(kt)  -> AP [128,1024] bf16 x^T / h_cat^T k-tile
      wh(jt)   -> AP [128,65]  bf16  [Wh block | ones col] (att matmul rhs)
      rcol(jt) -> AP [128,1] f32     exp(-0.8 t) column
      fcol(jt) -> AP [128,1] f32     exp(t) column
    emit_out(o_norm): consume the normalized [128, IB, FH] output (SBUF,
      dtype out_dt, layout [i, blk, f]).
    """
    work, psA, psO = pools["work"], pools["psA"], pools["psO"]
    workbig = pools["workbig"]

    # S_rep[p, i] = s[i] for all p (PE free-dim broadcast via replicated lhsT).
    sb_ps = psA.tile([128, N], F32, tag="big")
    for kt in range(KT):
        for ih in range(2):
            nc.tensor.matmul(
                sb_ps[:, ih * 512 : (ih + 1) * 512],
                inst["rep"](kt),
                inst["rhs"](kt)[:, ih * 512 : (ih + 1) * 512],
                start=(kt == 0),
                stop=(kt == KT - 1),
            )
    # G[p, i] = exp(0.8 * s[i])  (bf16 SBUF; one big ACT pass per instance)
    g16 = work.tile([128, N], BF, tag="g16")
    nc.scalar.activation(g16, sb_ps, AF.Exp, scale=0.8)

    # p'[j,i] = max(G[i], r[j]) * F[j], then mask multiply (in-place, batched).
    pT = workbig.tile([128, JT, N], BF, tag="pt")
    for jt in range(JT):
        nc.vector.tensor_scalar(
            pT[:, jt, :], g16, inst["rcol"](jt), inst["fcol"](jt), OP.max, OP.mult
        )
    for ih in range(2):
        half = slice(ih * (JT // 2), (ih + 1) * (JT // 2))
        nc.vector.tensor_tensor(
            pT[:, half, :], pT[:, half, :], maskT_sb[:, half, :], OP.mult
        )

    # O[i, blk, f] (+ rowsum col 64) -- p' chunks stationary, Wh block moving.
    # Two PSUM tiles of [128, 4, 65] (1040 B/partition) so no accumulation
    # region crosses a 2 KiB PSUM bank boundary; ib-outer order keeps one
    # accumulation group open at a time.
    HB = IB // 2
    ot_ps = [
        psO.tile([128, HB, FH + 1], F32, tag=f"ot{half}", name=f"ot{half}")
        for half in range(2)
    ]
    for ib in range(IB):
        for jt in range(JT):
            nc.tensor.matmul(
                ot_ps[ib // HB][:, ib % HB, :],
                pT[:, jt, ib * 128 : (ib + 1) * 128],
                inst["wh"](jt),
                start=(jt == 0),
                stop=(jt == JT - 1),
            )

    # Deferred tail: per-partition reciprocal of the rowsum columns, then the
    # normalization rides the PSUM->SBUF copies as an ACT per-partition scale.
    # Returned as a closure so the caller can emit it AFTER the next
    # instance's main body -- otherwise every engine idles on the chain.
    def finish():
        rsc = work.tile([128, IB], F32, tag="rsc")
        for half in range(2):
            nc.vector.tensor_copy(
                rsc[:, half * HB : (half + 1) * HB], ot_ps[half][:, :, FH]
            )
        rbc = work.tile([128, IB], F32, tag="rbc")
        nc.vector.reciprocal_approx_fast(out=rbc, in_=rsc)
        o_norm = work.tile([128, IB, FH], out_dt, tag="onrm")
        for ib in range(IB):
            nc.scalar.activation(
                o_norm[:, ib, :], ot_ps[ib // HB][:, ib % HB, 0:FH], AF.Copy,
                scale=rbc[:, ib : ib + 1],
            )
        emit_out(o_norm)

    return finish


def _build_nc():
    nc = bacc.Bacc()

    xT_d = nc.declare_dram_parameter("xT", [SPC, KT, 128, N], BF, isOutput=False)
    maskT_d = nc.declare_dram_parameter("maskT", [SPC, JT, 128, N], BF, isOutput=False)
    wbig1_d = nc.declare_dram_parameter("wbig1", [KT, 128, H * 65 + H], BF, isOutput=False)
    warep1_d = nc.declare_dram_parameter("warep1", [KT, 128, H * 128], BF, isOutput=False)
    wbig2_d = nc.declare_dram_parameter("wbig2", [KT, 128, 66], BF, isOutput=False)
    warep2_d = nc.declare_dram_parameter("warep2", [KT, 128, 128], BF, isOutput=False)
    ident_d = nc.declare_dram_parameter("ident", [128, 128], BF, isOutput=False)
    out_d = nc.declare_dram_parameter("out", [SPC, FOUT], F32, isOutput=True)

    with tile.TileContext(nc) as tc:
        with (
            tc.tile_pool(name="const", bufs=1) as constp,
            tc.tile_pool(name="samp", bufs=2) as samp,
            tc.tile_pool(name="workbig", bufs=2) as workbig,
            tc.tile_pool(name="work", bufs=3) as work,
            tc.tile_pool(name="tail", bufs=1) as tail,
            tc.tile_pool(name="psA", bufs=2, space="PSUM") as psA,
            tc.tile_pool(name="psO", bufs=2, space="PSUM") as psO,
        ):
            pools = {"work": work, "workbig": workbig, "psA": psA, "psO": psO}

            wbig1_sb = constp.tile([128, KT, H * 65 + H], BF)
            warep1_sb = constp.tile([128, KT, H * 128], BF)
            wbig2_sb = constp.tile([128, KT, 66], BF)
            warep2_sb = constp.tile([128, KT, 128], BF)
            ident_sb = constp.tile([128, 128], BF)
            nc.sync.dma_start(out=ident_sb, in_=ident_d[:, :])
            for kt in range(KT):
                nc.sync.dma_start(out=wbig1_sb[:, kt, :], in_=wbig1_d[kt])
                nc.sync.dma_start(out=warep1_sb[:, kt, :], in_=warep1_d[kt])
                nc.sync.dma_start(out=wbig2_sb[:, kt, :], in_=wbig2_d[kt])
                nc.sync.dma_start(out=warep2_sb[:, kt, :], in_=warep2_d[kt])
            ones128_sb = constp.tile([128, 1], BF)
            nc.vector.memset(ones128_sb, 1.0)

            pending_sample_tail = None
            for s in range(SPC):
                xT_sb = samp.tile([128, KT, N], BF, tag="xt")
                for kt in range(KT):
                    nc.sync.dma_start(out=xT_sb[:, kt, :], in_=xT_d[s, kt])
                maskT_sb = samp.tile([128, JT, N], BF, tag="mask")
                for jt in range(JT):
                    nc.sync.dma_start(out=maskT_sb[:, jt, :], in_=maskT_d[s, jt])

                # ---- L1 Wh for all 4 heads (+ t columns) ----
                whsb1 = samp.tile([128, JT, H * 65], BF, tag="whsb1")
                tc1 = samp.tile([128, JT, H], F32, tag="tc1")
                for jt in range(JT):
                    wm_ps = psA.tile([128, H * 65 + H], F32, tag="big")
                    for kt in range(KT):
                        nc.tensor.matmul(
                            wm_ps,
                            xT_sb[:, kt, jt * 128 : (jt + 1) * 128],
                            wbig1_sb[:, kt, :],
                            start=(kt == 0),
                            stop=(kt == KT - 1),
                        )
                    nc.scalar.copy(whsb1[:, jt, :], wm_ps[:, 0 : H * 65])
                    nc.vector.memset(whsb1[:, jt, FH : H * 65 : 65], 1.0)
                    nc.scalar.copy(tc1[:, jt, :], wm_ps[:, H * 65 : H * 65 + H])
                # r = exp(-0.8 t), F = exp(t) for all heads/j-tiles at once.
                rc1 = samp.tile([128, JT, H], F32, tag="rc1")
                fc1 = samp.tile([128, JT, H], F32, tag="fc1")
                nc.scalar.activation(rc1, tc1, AF.Exp, scale=-0.8)
                nc.scalar.activation(fc1, tc1, AF.Exp, scale=1.0)

                if pending_sample_tail is not None:
                    pending_sample_tail()
                    pending_sample_tail = None

                # ---- L1 attention, 4 heads -> h_cat^T (via PE transposes) ----
                hcatT = samp.tile([128, KT, N], BF, tag="hcat")

                pending = None
                for h in range(H):
                    def emit_l1(o_norm, h=h):
                        tp_ps = psA.tile([FH, IB, 128], BF, tag="big")
                        for ib in range(IB):
                            nc.tensor.transpose(
                                tp_ps[:, ib, :], o_norm[:, ib, :], ident_sb
                            )
                        dst = hcatT[(h % 2) * 64 : (h % 2) * 64 + 64, h // 2, :]
                        nc.scalar.copy(dst, tp_ps)

                    fin = _gat_instance(
                        nc,
                        pools,
                        maskT_sb,
                        {
                            "rep": lambda kt, h=h: warep1_sb[:, kt, h * 128 : (h + 1) * 128],
                            "rhs": lambda kt: xT_sb[:, kt, :],
                            "wh": lambda jt, h=h: whsb1[:, jt, h * 65 : (h + 1) * 65],
                            "rcol": lambda jt, h=h: rc1[:, jt, h : h + 1],
                            "fcol": lambda jt, h=h: fc1[:, jt, h : h + 1],
                        },
                        BF,
                        emit_l1,
                    )
                    if pending is not None:
                        pending()
                    pending = fin
                pending()

                # ---- L2 Wh ----
                whsb2 = samp.tile([128, JT, 65], BF, tag="whsb2")
                tc2 = samp.tile([128, JT, 1], F32, tag="tc2")
                for jt in range(JT):
                    wm_ps = psA.tile([128, 66], F32, tag="big")
                    for kt in range(KT):
                        nc.tensor.matmul(
                            wm_ps,
                            hcatT[:, kt, jt * 128 : (jt + 1) * 128],
                            wbig2_sb[:, kt, :],
                            start=(kt == 0),
                            stop=(kt == KT - 1),
                        )
                    nc.scalar.copy(whsb2[:, jt, 0:FOUT], wm_ps[:, 0:FOUT])
                    nc.vector.memset(whsb2[:, jt, FOUT : FOUT + 1], 1.0)
                    nc.scalar.copy(tc2[:, jt, :], wm_ps[:, 65:66])
                rc2 = samp.tile([128, JT, 1], F32, tag="rc2")
                fc2 = samp.tile([128, JT, 1], F32, tag="fc2")
                nc.scalar.activation(rc2, tc2, AF.Exp, scale=-0.8)
                nc.scalar.activation(fc2, tc2, AF.Exp, scale=1.0)

                # ---- L2 attention + elu + mean over nodes ----
                o2h = {}

                def emit_l2(o_norm, o2h=o2h):
                    o2h["o2n"] = o_norm

                fin2 = _gat_instance(
                    nc,
                    pools,
                    maskT_sb,
                    {
                        "rep": lambda kt: warep2_sb[:, kt, :],
                        "rhs": lambda kt: hcatT[:, kt, :],
                        "wh": lambda jt: whsb2[:, jt, :],
                        "rcol": lambda jt: rc2[:, jt, :],
                        "fcol": lambda jt: fc2[:, jt, :],
                    },
                    F32,
                    emit_l2,
                )
                def sample_tail(s=s, fin2=fin2, o2h=o2h):
                    fin2()
                    o2n = o2h["o2n"]
                    # elu(x) = relu(x) + min(exp(x), 1) - 1; the constant -1
                    # is folded into the post-reduce scale: mean = sum/N - 1.
                    ex = tail.tile([128, IB, FH], F32, tag="ex")
                    nc.scalar.activation(ex, o2n, AF.Exp)
                    bmax = tail.tile([128, IB, FH], F32, tag="bmax")
                    nc.scalar.activation(bmax, o2n, AF.Relu)
                    eluv = tail.tile([128, IB, FH], BF, tag="eluv")
                    nc.vector.scalar_tensor_tensor(
                        eluv, ex, 1.0, bmax, OP.min, OP.add
                    )
                    # mean over nodes: i is the partition dim, contract on PE.
                    mean_ps = psA.tile([FH, 1], F32, tag="big")
                    for ib in range(IB):
                        nc.tensor.matmul(
                            mean_ps,
                            eluv[:, ib, :],
                            ones128_sb,
                            start=(ib == 0),
                            stop=(ib == IB - 1),
                        )
                    outc = tail.tile([FH, 1], F32, tag="outc")
                    nc.vector.tensor_scalar(outc, mean_ps, 1.0 / N, -1.0, OP.mult, OP.add)
                    nc.sync.dma_start(
                        out=out_d[s].rearrange("(f a) -> f a", a=1), in_=outc
                    )

                pending_sample_tail = sample_tail

            pending_sample_tail()

    nc.finalize()
    return nc


_NC_CACHE = None


def _prep_host(x, adj, W_heads, a_heads, W_out, a_out):
    xT = np.ascontiguousarray(np.asarray(x, np.float32).transpose(0, 2, 1)).astype(BF16)
    xT = xT.reshape(B, KT, 128, N)
    maskT = (np.asarray(adj) > 0).transpose(0, 2, 1).astype(BF16)  # [B, j, i]
    maskT = np.ascontiguousarray(maskT).reshape(B, JT, 128, N)

    W_heads = np.asarray(W_heads, np.float32)
    a_heads = np.asarray(a_heads, np.float32)
    W_out = np.asarray(W_out, np.float32)
    a_out = np.asarray(a_out, np.float32)

    wbig1 = np.zeros((FIN, H * 65 + H), dtype=np.float32)
    warep1 = np.zeros((FIN, H * 128), dtype=np.float32)
    for h in range(H):
        Wh_ = W_heads[h]
        wbig1[:, h * 65 : h * 65 + FH] = Wh_
        wbig1[:, H * 65 + h] = Wh_ @ a_heads[h, FH:, 0]
        warep1[:, h * 128 : (h + 1) * 128] = (Wh_ @ a_heads[h, :FH, 0])[:, None]
    wbig2 = np.zeros((FIN, 66), dtype=np.float32)
    wbig2[:, 0:FOUT] = W_out
    wbig2[:, 65] = W_out @ a_out[FOUT:, 0]
    warep2 = np.repeat((W_out @ a_out[:FOUT, 0])[:, None], 128, axis=1)

    shared = {
        "wbig1": wbig1.astype(BF16).reshape(KT, 128, H * 65 + H),
        "warep1": warep1.astype(BF16).reshape(KT, 128, H * 128),
        "wbig2": wbig2.astype(BF16).reshape(KT, 128, 66),
        "warep2": warep2.astype(BF16).reshape(KT, 128, 128),
        "ident": np.eye(128, dtype=np.float32).astype(BF16),
    }
    in_maps = []
    for c in range(NCORES):
        sl = slice(c * SPC, (c + 1) * SPC)
        m = {"xT": np.ascontiguousarray(xT[sl]), "maskT": np.ascontiguousarray(maskT[sl])}
        m.update(shared)
        in_maps.append(m)
    return in_maps


def kernel(x, adj, W_heads, a_heads, W_out, a_out, _trace=False):
    global _NC_CACHE
    if _NC_CACHE is None:
        _NC_CACHE = _build_nc()
    nc = _NC_CACHE
    in_maps = _prep_host(x, adj, W_heads, a_heads, W_out, a_out)
    res = run_bass_kernel_spmd(nc, in_maps, core_ids=list(range(NCORES)), trace=_trace)
    out = np.concatenate([res.results[c]["out"] for c in range(NCORES)], axis=0)
    if _trace:
        kernel._last_results = res
    return out.astype(np.float32)
